# revision 18
# baseline (speedup 1.0000x reference)
"""Causal self-attention (B=4, T=2048, C=1024, H=16, D=64) on 8 trn2 NeuronCores.

Sharding: core c = (batch b = c//2, head-group g = c%2). Megatron-style within a
batch: each core computes 8 heads' q/k/v (column-parallel) and a row-parallel
partial out-projection. Host sums the two partials per batch and adds the
rank-1 bias term (bo + bv @ wo) -- valid because softmax rows sum to 1, so v's
bias never needs to enter the kernel.

Schedule (v2): fine-grained software pipelining. The q/k projection for pair t
of chunk jq ("qk unit") is issued as PE fill work inside the PREVIOUS pair's
attention (which is ScalarE/exp-bound), v-projection units for chunk jq+1 and
out-projection m-chunks are likewise interleaved as fills at attention tile
milestones. The tail normalize uses a PE K=1 ones-outer-product to broadcast
1/Z (no DRAM round trip), and y is written back in bf16 (summed in f64 host-
side; rel-err impact ~4e-4).
"""
import numpy as np
import ml_dtypes
from collections import deque

import concourse.tile as tile
from concourse import bacc, mybir
from concourse.bass_utils import run_bass_kernel_spmd

BF16 = ml_dtypes.bfloat16
F32 = mybir.dt.float32
BT16 = mybir.dt.bfloat16
AF = mybir.ActivationFunctionType
ALU = mybir.AluOpType

B, T, C, H, D = 4, 2048, 1024, 16, 64
G = 2              # head groups (cores per batch)
HL = H // G        # heads per core = 8
HD = HL * D        # local head dims = 512
NP = 4             # head pairs per core
NJQ = T // 512     # q chunks of 512 = 4
NIK = T // 128     # k tiles of 128 = 16
KC = C // 128      # contraction chunks = 8

_CACHED = {}


def _build():
    nc = bacc.Bacc("TRN2", debug=False)
    xT = nc.dram_tensor("xT", [C, T], BT16, kind="ExternalInput").ap()
    wqk = nc.dram_tensor("wqk", [C, 2 * HD], BT16, kind="ExternalInput").ap()
    wv = nc.dram_tensor("wv", [C, HD], BT16, kind="ExternalInput").ap()
    wo = nc.dram_tensor("wo", [HD, C], BT16, kind="ExternalInput").ap()
    bq = nc.dram_tensor("bq", [128, NP], F32, kind="ExternalInput").ap()
    bk = nc.dram_tensor("bk", [128, NP], F32, kind="ExternalInput").ap()
    masks = nc.dram_tensor("masks", [128, 4, 512], BT16, kind="ExternalInput").ap()
    rcp_dram = nc.dram_tensor("rcp_dram", [NJQ, 8, 512], BT16).ap()
    y = nc.dram_tensor("y", [T, C], BT16, kind="ExternalOutput").ap()

    with tile.TileContext(nc) as tc:
        with (
            tc.tile_pool(name="consts", bufs=1) as consts,
            tc.tile_pool(name="xt", bufs=2) as xtp,
            tc.tile_pool(name="qk", bufs=1) as qkp,
            tc.tile_pool(name="vp", bufs=1) as vp,
            tc.tile_pool(name="otp", bufs=1) as otp,
            tc.tile_pool(name="pt", bufs=6) as ptp,
            tc.tile_pool(name="ptmp", bufs=3) as ptmpp,
            tc.tile_pool(name="zn", bufs=3) as znp,
            tc.tile_pool(name="yst", bufs=4) as ystp,
            tc.tile_pool(name="ps", bufs=2, space="PSUM") as ps,
        ):
            # ---- constants ----
            bq_sb = consts.tile([128, NP], F32, tag="bq")
            nc.sync.dma_start(bq_sb, bq)
            bk_sb = consts.tile([128, NP], F32, tag="bk")
            nc.sync.dma_start(bk_sb, bk)
            masks_sb = consts.tile([128, 4, 512], BT16, tag="masks")
            nc.sync.dma_start(masks_sb, masks)
            wqk_sb = consts.tile([128, KC, 2 * HD], BT16, tag="wqk")
            wqk_r = wqk.rearrange("(k p) c -> p k c", p=128)

            def dma_xt(jt):
                xt = xtp.tile([128, KC, 512], BT16, tag="xt", bufs=3, name=f"xt{jt}")
                xr = xT[:, jt * 512:(jt + 1) * 512].rearrange("(k p) t -> p k t", p=128)
                for k in range(KC):
                    (nc.sync if (jt == 0 or k % 2 == 0) else nc.gpsimd).dma_start(
                        xt[:, k, :], xr[:, k, :]
                    )
                return xt

            # preload the Exp activation table while the first DMAs fly
            dummy_in = consts.tile([1, 16], F32, tag="dummy_in")
            nc.vector.memset(dummy_in, 0.0)
            dummy_out = consts.tile([1, 16], F32, tag="dummy_out")
            nc.scalar.activation(dummy_out, dummy_in, AF.Exp)
            # startup DMA priority across all three DMA queues:
            # sync: xt0; gpsimd: even wqk chunks; scalar: odd wqk chunks
            xts = {0: dma_xt(0)}
            for k in range(0, KC, 2):
                nc.gpsimd.dma_start(wqk_sb[:, k, :], wqk_r[:, k, :])
                nc.scalar.dma_start(wqk_sb[:, k + 1, :], wqk_r[:, k + 1, :])
            wv_sb = consts.tile([128, KC, HD], BT16, tag="wv")
            wv_r = wv.rearrange("(k p) c -> p k c", p=128)
            for k in range(0, KC, 2):
                nc.gpsimd.dma_start(wv_sb[:, k, :], wv_r[:, k, :])
                nc.scalar.dma_start(wv_sb[:, k + 1, :], wv_r[:, k + 1, :])
            wo_sb = consts.tile([128, NP, C], BT16, tag="wo")
            nc.gpsimd.dma_start(wo_sb, wo.rearrange("(t p) c -> p t c", p=128))
            ones_bf = consts.tile([1, 64], BT16, tag="ones_bf")
            nc.vector.memset(ones_bf, 1.0)

            # ---- persistent activations ----
            qT = [qkp.tile([128, T], BT16, tag=f"qT{t}", name=f"qT{t}") for t in range(NP)]
            kT = [qkp.tile([128, T], BT16, tag=f"kT{t}", name=f"kT{t}") for t in range(NP)]
            v_sb = [vp.tile([128, HL * 65], BT16, tag=f"v{i}", name=f"v{i}") for i in range(NIK)]
            oT = [otp.tile([128, T], BT16, tag=f"oT{t}", name=f"oT{t}") for t in range(NP)]

            # ---- phase-1 units ----
            def qk_q(jt, t, xt):
                p = ps.tile([128, 512], F32, tag="fp", bufs=2, name=f"pq{jt}_{t}")
                for k in range(KC):
                    nc.tensor.matmul(
                        p, wqk_sb[:, k, t * 128:(t + 1) * 128], xt[:, k, :],
                        start=(k == 0), stop=(k == KC - 1),
                    )
                if jt == 1 or (jt == 0 and t >= 2):
                    nc.scalar.activation(
                        qT[t][:, jt * 512:(jt + 1) * 512], p,
                        AF.Identity, bias=bq_sb[:, t:t + 1], scale=0.125,
                    )
                else:
                    nc.vector.tensor_scalar(
                        qT[t][:, jt * 512:(jt + 1) * 512], p,
                        0.125, bq_sb[:, t:t + 1], ALU.mult, ALU.add,
                    )

            def qk_k(jt, t, xt):
                p = ps.tile([128, 512], F32, tag="fp", bufs=2, name=f"pk{jt}_{t}")
                for k in range(KC):
                    nc.tensor.matmul(
                        p, wqk_sb[:, k, HD + t * 128:HD + (t + 1) * 128], xt[:, k, :],
                        start=(k == 0), stop=(k == KC - 1),
                    )
                if jt == 1 or (jt == 0 and t >= 2):
                    nc.scalar.activation(
                        kT[t][:, jt * 512:(jt + 1) * 512], p,
                        AF.Identity, bias=bk_sb[:, t:t + 1], scale=1.0,
                    )
                else:
                    nc.vector.tensor_scalar_add(
                        kT[t][:, jt * 512:(jt + 1) * 512], p, bk_sb[:, t:t + 1]
                    )

            def v_unit(jt, s, xt):
                ik = jt * 4 + s
                p = ps.tile([128, 512], F32, tag="fp", bufs=2, name=f"pv{ik}")
                for k in range(KC):
                    nc.tensor.matmul(
                        p, xt[:, k, s * 128:(s + 1) * 128], wv_sb[:, k, :],
                        start=(k == 0), stop=(k == KC - 1),
                    )
                vg = v_sb[ik].rearrange("p (h c) -> p h c", c=65)
                nc.vector.tensor_copy(
                    vg[:, :, 0:64], p.rearrange("p (h c) -> p h c", c=64)
                )
                nc.vector.memset(vg[:, :, 64:65], 1.0)

            # ---- out-projection (m, n) sub-chunk; one merged y DMA per m ----
            ys_tiles = {}

            def phase3_n(m, n, alt=False):
                p = ps.tile([128, 512], F32, tag="fp", bufs=2, name=f"py{m}_{n}")
                for t in range(NP):
                    nc.tensor.matmul(
                        p, oT[t][:, m * 128:(m + 1) * 128],
                        wo_sb[:, t, n * 512:(n + 1) * 512],
                        start=(t == 0), stop=(t == NP - 1),
                    )
                if n == 0:
                    ys_tiles[m] = ystp.tile([128, 1024], BT16, tag="y", name=f"ys{m}")
                ys = ys_tiles[m]
                if alt:
                    nc.scalar.copy(ys[:, n * 512:(n + 1) * 512], p)
                else:
                    nc.vector.tensor_copy(ys[:, n * 512:(n + 1) * 512], p)
                if n == 1:
                    nc.gpsimd.dma_start(y[m * 128:(m + 1) * 128, :], ys)

            # ---- attention ----
            def av(t, ik, nik, pts, o_ps):
                pt, c0 = pts[ik]
                ptg = pt.rearrange("p (h q) -> p h q", q=512)
                for hh in range(2):
                    h = 2 * t + hh
                    nc.tensor.matmul(
                        o_ps[hh][:, c0:512], v_sb[ik][:, h * 65:h * 65 + 65],
                        ptg[:, hh, c0:512],
                        start=(ik == 0), stop=(ik == nik - 1),
                    )

            def attention(t, jq, fills):
                nik = 4 * jq + 4
                o_ps = [
                    ps.tile([65, 512], F32, tag="ot", bufs=2, name=f"ops{t}_{jq}_{_h}")
                    for _h in range(2)
                ]
                pts = {}
                for ik in range(nik):
                    d = ik - 4 * jq
                    c0 = 128 * d if d > 0 else 0   # first potentially-valid column
                    st = ps.tile([128, 1024], F32, tag="st", name=f"st{t}_{jq}_{ik}")
                    stg = st.rearrange("p (h q) -> p h q", q=512)
                    for hh in range(2):
                        r = slice(hh * 64, hh * 64 + 64)
                        nc.tensor.matmul(
                            stg[:, hh, c0:512],
                            kT[t][r, ik * 128:(ik + 1) * 128],
                            qT[t][r, jq * 512 + c0:(jq + 1) * 512],
                            start=True, stop=True,
                        )
                    pt = ptp.tile([128, 1024], BT16, tag="pt", name=f"pt{t}_{jq}_{ik}")
                    ptg = pt.rearrange("p (h q) -> p h q", q=512)
                    if d >= 0:
                        ptm = ptmpp.tile([128, 1024], BT16, tag="ptmp", name=f"ptm{t}_{jq}_{ik}")
                        ptmg = ptm.rearrange("p (h q) -> p h q", q=512)
                        nc.scalar.activation(ptmg[:, :, c0:512], stg[:, :, c0:512], AF.Exp)
                        for hh in range(2):
                            nc.vector.tensor_mul(
                                ptg[:, hh, c0:512],
                                ptmg[:, hh, c0:512],
                                masks_sb[:, d, c0:512],
                            )
                    else:
                        nc.scalar.activation(pt, st, AF.Exp)
                    pts[ik] = (pt, c0)
                    if fills and (ik % 3 == 2 or (jq == 0 and ik >= 1)):
                        fills.popleft()()
                    if ik > 0:
                        av(t, ik - 1, nik, pts, o_ps)
                av(t, nik - 1, nik, pts, o_ps)
                # evict Z row + unnormalized O^T, freeing the PSUM accumulators
                out_h = []
                for hh in range(2):
                    ouz = znp.tile([65, 512], F32, tag="ouz", bufs=6, name=f"oz{t}_{jq}_{hh}")
                    nc.vector.tensor_copy(ouz, o_ps[hh])
                    out_h.append(ouz)
                while fills:
                    fills.popleft()()
                return out_h

            import concourse.bass as bass_mod

            def normalize_a(t, jq, evicted):
                # Stage A: kick off the 1/Z DRAM-broadcast chain. Pack both
                # heads' Z rows [1,512] as [8,64] each -> one [16,64]
                # reciprocal (64 elems/lane), then broadcast 1/Z via a DRAM
                # round-trip (partition-step-0 DMA reads are legal from DRAM).
                zb = znp.tile([16, 64], F32, tag="zb", bufs=2, name=f"zb{t}_{jq}")
                for hh in range(2):
                    ouz = evicted[hh]
                    nc.sync.dma_start(
                        zb[8 * hh:8 * hh + 8, :],
                        ouz[64:65, :].rearrange("o (p q) -> o p q", p=8),
                    )
                rcp = znp.tile([16, 64], F32, tag="rcpb", bufs=2, name=f"rcp{t}_{jq}")
                nc.vector.reciprocal(rcp, zb)
                rcp16 = znp.tile([16, 64], BT16, tag="rcp16b", bufs=2, name=f"rcp16{t}_{jq}")
                nc.vector.tensor_copy(rcp16, rcp)
                nc.sync.dma_start(
                    rcp_dram[jq, 2 * t:2 * t + 2, :].rearrange("h (p q) -> (h p) q", p=8),
                    rcp16,
                )
                bcs = []
                for hh in range(2):
                    bc_sb = znp.tile([64, 512], BT16, tag="bc_sb", bufs=4, name=f"bs{t}_{jq}_{hh}")
                    src = rcp_dram[jq, 2 * t + hh, :]
                    bcast = bass_mod.AP(
                        tensor=src.tensor, offset=src.offset,
                        ap=[[0, 64]] + [list(a) for a in src.ap],
                    )
                    nc.sync.dma_start(bc_sb, bcast)
                    bcs.append(bc_sb)
                return bcs

            def normalize_b(t, jq, evicted, bcs):
                # Stage B (one slot later, after the broadcast landed): scale
                # O^T by 1/Z.
                qs2 = slice(jq * 512, (jq + 1) * 512)
                nc.vector.tensor_mul(oT[t][0:64, qs2], evicted[0][0:64, :], bcs[0])
                tmp = znp.tile([64, 512], BT16, tag="tmp_o", bufs=2, name=f"tm{t}_{jq}")
                nc.vector.tensor_mul(tmp, evicted[1][0:64, :], bcs[1])
                nc.gpsimd.dma_start(oT[t][64:128, qs2], tmp)

            # ---- main schedule ----
            pendA = deque()   # attentions awaiting stage-A normalize
            pendB = deque()   # awaiting stage-B (muls), one slot later
            p3q = deque()
            qk_q(0, 0, xts[0])
            qk_k(0, 0, xts[0])
            xts[1] = dma_xt(1)
            for s in range(3):
                v_unit(0, s, xts[0])
            for jq in range(NJQ):
                for t in range(NP):
                    if t == 1 and jq >= 1 and jq + 1 < NJQ:
                        xts[jq + 1] = dma_xt(jq + 1)
                    fills = deque()
                    if jq == 0 and t == 0:
                        fills.append(lambda: v_unit(0, 3, xts[0]))
                    if t < NP - 1:
                        fills.append(lambda jt=jq, tt=t + 1: qk_q(jt, tt, xts[jt]))
                        fills.append(lambda jt=jq, tt=t + 1: qk_k(jt, tt, xts[jt]))
                    elif jq + 1 < NJQ:
                        fills.append(lambda jt=jq + 1: qk_q(jt, 0, xts[jt]))
                        fills.append(lambda jt=jq + 1: qk_k(jt, 0, xts[jt]))
                    if p3q:
                        m = p3q.popleft()
                        fills.append(lambda mm=m: phase3_n(mm, 0))
                        fills.append(lambda mm=m: phase3_n(mm, 1))
                    if t >= 2 and jq + 1 < NJQ:
                        s0 = 2 * (t - 2)
                        fills.append(lambda jt=jq + 1, s=s0: v_unit(jt, s, xts[jt]))
                        fills.append(lambda jt=jq + 1, s=s0 + 1: v_unit(jt, s, xts[jt]))
                    ev = attention(t, jq, fills)
                    if pendA:
                        ta, ja, eva = pendA.popleft()
                        bcs = normalize_a(ta, ja, eva)
                        pendB.append((ta, ja, eva, bcs))
                    if len(pendB) >= 2:
                        tb, jb, evb, bcsb = pendB.popleft()
                        normalize_b(tb, jb, evb, bcsb)
                        if tb == NP - 1:
                            p3q.extend(range(4 * jb, 4 * jb + 4))
                    pendA.append((t, jq, ev))

            # ---- tail ----
            # finish (2,3): its broadcast landed during attention(3,3)
            ta, ja, eva = pendA.popleft()      # (3,3) -> handled via PE path
            tb, jb, evb, bcsb = pendB.popleft()
            normalize_b(tb, jb, evb, bcsb)
            # (3,3): 1/Z via SBUF gather + K=1 ones outer-product on the PE —
            # no DRAM round trip; reserved out-proj chunks keep the PE warm.
            ouz0, ouz1 = eva
            zb = znp.tile([16, 64], F32, tag="zb", bufs=2, name="zb_tail")
            for hh in range(2):
                nc.sync.dma_start(
                    zb[8 * hh:8 * hh + 8, :],
                    eva[hh][64:65, :].rearrange("o (p q) -> o p q", p=8),
                )
            rcp = znp.tile([16, 64], F32, tag="rcpb", bufs=2, name="rcp_tail")
            nc.vector.reciprocal(rcp, zb)
            rcp16 = znp.tile([16, 64], BT16, tag="rcp16b", bufs=2, name="rcp16_tail")
            nc.vector.tensor_copy(rcp16, rcp)
            rcpln = znp.tile([1, 1024], BT16, tag="rcpln", bufs=1, name="rcpln")
            nc.sync.dma_start(
                rcpln.rearrange("o (p q) -> o p q", p=16), rcp16
            )
            while p3q:     # PE fill while the 1/Z chain completes
                m = p3q.popleft()
                phase3_n(m, 0)
                phase3_n(m, 1)
            bc_pair = ps.tile([128, 1024], F32, tag="st", name="bc_pair")
            nc.tensor.matmul(bc_pair[0:64, 0:512], ones_bf, rcpln[:, 0:512],
                             start=True, stop=True)
            nc.tensor.matmul(bc_pair[0:64, 512:1024], ones_bf, rcpln[:, 512:1024],
                             start=True, stop=True)
            for mi in range(4):
                cs = slice(mi * 128, (mi + 1) * 128)
                gs = slice(ja * 512 + mi * 128, ja * 512 + (mi + 1) * 128)
                nc.vector.tensor_mul(oT[ta][0:64, gs], ouz0[0:64, cs], bc_pair[0:64, cs])
                tmp = znp.tile([64, 128], BT16, tag="tmp_os", bufs=4, name=f"tms{mi}")
                nc.vector.tensor_mul(tmp, ouz1[0:64, cs], bc_pair[0:64, 512 + mi * 128:512 + (mi + 1) * 128])
                nc.sync.dma_start(oT[ta][64:128, gs], tmp)
                phase3_n(4 * ja + mi, 0, alt=True)
                phase3_n(4 * ja + mi, 1, alt=True)

    nc.compile()
    return nc


def _host_prep(x, wq, bq, wk, bk, wv, wo):
    masks_np = np.zeros((128, 4, 512), dtype=BF16)
    qn = np.arange(512)[None, :]
    kn = np.arange(128)[:, None]
    for d in range(4):
        masks_np[:, d, :] = (qn >= kn + 128 * d).astype(BF16)

    per_g = []
    for g in range(G):
        cs = slice(g * HD, (g + 1) * HD)
        per_g.append({
            "wqk": np.ascontiguousarray(
                np.concatenate([wq[:, cs], wk[:, cs]], axis=1)
            ).astype(BF16),
            "wv": np.ascontiguousarray(wv[:, cs]).astype(BF16),
            "wo": np.ascontiguousarray(wo[cs, :]).astype(BF16),
            "bq": np.ascontiguousarray((bq[cs] / 8.0).reshape(NP, 128).T).astype(np.float32),
            "bk": np.ascontiguousarray(bk[cs].reshape(NP, 128).T).astype(np.float32),
            "masks": masks_np,
        })
    in_maps = []
    for c in range(8):
        b, g = divmod(c, G)
        m = dict(per_g[g])
        m["xT"] = np.ascontiguousarray(x[b].T).astype(BF16)
        in_maps.append(m)
    return in_maps


def kernel(x, wq, bq, wk, bk, wv, bv, wo, bo):
    x = np.asarray(x, dtype=np.float32)
    wq = np.asarray(wq, dtype=np.float32)
    bq = np.asarray(bq, dtype=np.float32)
    wk = np.asarray(wk, dtype=np.float32)
    bk = np.asarray(bk, dtype=np.float32)
    wv = np.asarray(wv, dtype=np.float32)
    bv = np.asarray(bv, dtype=np.float32)
    wo = np.asarray(wo, dtype=np.float32)
    bo = np.asarray(bo, dtype=np.float32)

    if "nc" not in _CACHED:
        _CACHED["nc"] = _build()
    nc = _CACHED["nc"]

    in_maps = _host_prep(x, wq, bq, wk, bk, wv, wo)
    res = run_bass_kernel_spmd(nc, in_maps, core_ids=list(range(8)))

    const_row = (bo.astype(np.float64) + bv.astype(np.float64) @ wo.astype(np.float64))
    out = np.empty((B, T, C), dtype=np.float32)
    for b in range(B):
        acc = res.results[2 * b]["y"].astype(np.float64)
        acc += res.results[2 * b + 1]["y"].astype(np.float64)
        acc += const_row[None, :]
        out[b] = acc.astype(np.float32)
    return out


# revision 22
# speedup vs baseline: 1.0235x; 1.0235x over previous
"""Causal self-attention (B=4, T=2048, C=1024, H=16, D=64) on 8 trn2 NeuronCores.

Sharding: core c = (batch b = c//2, head-group g = c%2). Megatron-style within a
batch: each core computes 8 heads' q/k/v (column-parallel) and a row-parallel
partial out-projection. Host sums the two partials per batch and adds the
rank-1 bias term (bo + bv @ wo) -- valid because softmax rows sum to 1, so v's
bias never needs to enter the kernel.

Schedule (v2): fine-grained software pipelining. The q/k projection for pair t
of chunk jq ("qk unit") is issued as PE fill work inside the PREVIOUS pair's
attention (which is ScalarE/exp-bound), v-projection units for chunk jq+1 and
out-projection m-chunks are likewise interleaved as fills at attention tile
milestones. The tail normalize uses a PE K=1 ones-outer-product to broadcast
1/Z (no DRAM round trip), and y is written back in bf16 (summed in f64 host-
side; rel-err impact ~4e-4).
"""
import numpy as np
import ml_dtypes
from collections import deque

import concourse.tile as tile
from concourse import bacc, mybir
from concourse.bass_utils import run_bass_kernel_spmd

BF16 = ml_dtypes.bfloat16
F32 = mybir.dt.float32
BT16 = mybir.dt.bfloat16
AF = mybir.ActivationFunctionType
ALU = mybir.AluOpType

B, T, C, H, D = 4, 2048, 1024, 16, 64
G = 2              # head groups (cores per batch)
HL = H // G        # heads per core = 8
HD = HL * D        # local head dims = 512
NP = 4             # head pairs per core
NJQ = T // 512     # q chunks of 512 = 4
NIK = T // 128     # k tiles of 128 = 16
KC = C // 128      # contraction chunks = 8

_CACHED = {}


def _build():
    nc = bacc.Bacc("TRN2", debug=False)
    xT = nc.dram_tensor("xT", [C, T], BT16, kind="ExternalInput").ap()
    wqk = nc.dram_tensor("wqk", [C, 2 * HD], BT16, kind="ExternalInput").ap()
    wv = nc.dram_tensor("wv", [C, HD], BT16, kind="ExternalInput").ap()
    wo = nc.dram_tensor("wo", [HD, C], BT16, kind="ExternalInput").ap()
    bq = nc.dram_tensor("bq", [128, NP], F32, kind="ExternalInput").ap()
    bk = nc.dram_tensor("bk", [128, NP], F32, kind="ExternalInput").ap()
    masks = nc.dram_tensor("masks", [128, 4, 512], BT16, kind="ExternalInput").ap()
    rcp_dram = nc.dram_tensor("rcp_dram", [NJQ, 8, 512], BT16).ap()
    y = nc.dram_tensor("y", [T, C], BT16, kind="ExternalOutput").ap()

    with tile.TileContext(nc) as tc:
        with (
            tc.tile_pool(name="consts", bufs=1) as consts,
            tc.tile_pool(name="xt", bufs=2) as xtp,
            tc.tile_pool(name="qk", bufs=1) as qkp,
            tc.tile_pool(name="vp", bufs=1) as vp,
            tc.tile_pool(name="otp", bufs=1) as otp,
            tc.tile_pool(name="pt", bufs=6) as ptp,
            tc.tile_pool(name="ptmp", bufs=3) as ptmpp,
            tc.tile_pool(name="zn", bufs=3) as znp,
            tc.tile_pool(name="yst", bufs=4) as ystp,
            tc.tile_pool(name="ps", bufs=2, space="PSUM") as ps,
        ):
            # ---- constants ----
            bq_sb = consts.tile([128, NP], F32, tag="bq")
            nc.sync.dma_start(bq_sb, bq)
            bk_sb = consts.tile([128, NP], F32, tag="bk")
            nc.sync.dma_start(bk_sb, bk)
            masks_sb = consts.tile([128, 4, 512], BT16, tag="masks")
            wqk_sb = consts.tile([128, KC, 2 * HD], BT16, tag="wqk")
            wqk_r = wqk.rearrange("(k p) c -> p k c", p=128)

            def dma_xt(jt):
                # 2-chunk (256 KB) pair DMAs; mid-kernel chunks stay off the
                # sync queue, which is reserved for the 1/Z broadcast chains.
                xt = xtp.tile([128, KC, 512], BT16, tag="xt", bufs=3, name=f"xt{jt}")
                xr = xT[:, jt * 512:(jt + 1) * 512].rearrange("(k p) t -> p k t", p=128)
                for k in range(0, KC, 2):
                    if jt == 0:
                        eng = nc.sync
                    else:
                        eng = nc.gpsimd if (k // 2) % 2 == 0 else nc.sync
                    eng.dma_start(xt[:, k:k + 2, :], xr[:, k:k + 2, :])
                return xt

            # startup DMA priority across all three DMA queues:
            # sync: xt0 pairs; gpsimd: even wqk chunks; scalar: odd wqk chunks
            dummy_in = consts.tile([1, 16], F32, tag="dummy_in")
            nc.vector.memset(dummy_in, 0.0)
            xts = {0: dma_xt(0)}
            wv_sb = consts.tile([128, KC, HD], BT16, tag="wv")
            wv_r = wv.rearrange("(k p) c -> p k c", p=128)
            for k in range(0, KC, 2):
                nc.gpsimd.dma_start(wqk_sb[:, k, :], wqk_r[:, k, :])
                nc.scalar.dma_start(wqk_sb[:, k + 1, :], wqk_r[:, k + 1, :])
            # preload the Exp activation table while the weight DMAs fly
            dummy_out = consts.tile([1, 16], F32, tag="dummy_out")
            nc.scalar.activation(dummy_out, dummy_in, AF.Exp)
            nc.gpsimd.dma_start(masks_sb, masks)
            for k in range(0, KC, 2):
                nc.gpsimd.dma_start(wv_sb[:, k, :], wv_r[:, k, :])
                nc.scalar.dma_start(wv_sb[:, k + 1, :], wv_r[:, k + 1, :])
            wo_sb = consts.tile([128, NP, C], BT16, tag="wo")
            nc.gpsimd.dma_start(wo_sb, wo.rearrange("(t p) c -> p t c", p=128))
            ones_bf = consts.tile([1, 64], BT16, tag="ones_bf")
            nc.vector.memset(ones_bf, 1.0)

            # ---- persistent activations ----
            qT = [qkp.tile([128, T], BT16, tag=f"qT{t}", name=f"qT{t}") for t in range(NP)]
            kT = [qkp.tile([128, T], BT16, tag=f"kT{t}", name=f"kT{t}") for t in range(NP)]
            v_sb = [vp.tile([128, HL * 65], BT16, tag=f"v{i}", name=f"v{i}") for i in range(NIK)]
            oT = [otp.tile([128, T], BT16, tag=f"oT{t}", name=f"oT{t}") for t in range(NP)]

            # ---- phase-1 units ----
            def qk_q(jt, t, xt):
                p = ps.tile([128, 512], F32, tag="fp", bufs=2, name=f"pq{jt}_{t}")
                for k in range(KC):
                    nc.tensor.matmul(
                        p, wqk_sb[:, k, t * 128:(t + 1) * 128], xt[:, k, :],
                        start=(k == 0), stop=(k == KC - 1),
                    )
                if jt == 1 or (jt == 0 and t >= 2):
                    nc.scalar.activation(
                        qT[t][:, jt * 512:(jt + 1) * 512], p,
                        AF.Identity, bias=bq_sb[:, t:t + 1], scale=0.125,
                    )
                else:
                    nc.vector.tensor_scalar(
                        qT[t][:, jt * 512:(jt + 1) * 512], p,
                        0.125, bq_sb[:, t:t + 1], ALU.mult, ALU.add,
                    )

            def qk_k(jt, t, xt):
                p = ps.tile([128, 512], F32, tag="fp", bufs=2, name=f"pk{jt}_{t}")
                for k in range(KC):
                    nc.tensor.matmul(
                        p, wqk_sb[:, k, HD + t * 128:HD + (t + 1) * 128], xt[:, k, :],
                        start=(k == 0), stop=(k == KC - 1),
                    )
                if jt == 1 or (jt == 0 and t >= 2):
                    nc.scalar.activation(
                        kT[t][:, jt * 512:(jt + 1) * 512], p,
                        AF.Identity, bias=bk_sb[:, t:t + 1], scale=1.0,
                    )
                else:
                    nc.vector.tensor_scalar_add(
                        kT[t][:, jt * 512:(jt + 1) * 512], p, bk_sb[:, t:t + 1]
                    )

            def v_unit(jt, s, xt):
                ik = jt * 4 + s
                p = ps.tile([128, 512], F32, tag="fp", bufs=2, name=f"pv{ik}")
                for k in range(KC):
                    nc.tensor.matmul(
                        p, xt[:, k, s * 128:(s + 1) * 128], wv_sb[:, k, :],
                        start=(k == 0), stop=(k == KC - 1),
                    )
                vg = v_sb[ik].rearrange("p (h c) -> p h c", c=65)
                nc.vector.tensor_copy(
                    vg[:, :, 0:64], p.rearrange("p (h c) -> p h c", c=64)
                )
                nc.vector.memset(vg[:, :, 64:65], 1.0)

            # ---- out-projection (m, n) sub-chunk; one merged y DMA per m ----
            ys_tiles = {}

            def phase3_n(m, n, alt=False):
                p = ps.tile([128, 512], F32, tag="fp", bufs=2, name=f"py{m}_{n}")
                for t in range(NP):
                    nc.tensor.matmul(
                        p, oT[t][:, m * 128:(m + 1) * 128],
                        wo_sb[:, t, n * 512:(n + 1) * 512],
                        start=(t == 0), stop=(t == NP - 1),
                    )
                if n == 0:
                    ys_tiles[m] = ystp.tile([128, 1024], BT16, tag="y", name=f"ys{m}")
                ys = ys_tiles[m]
                if alt:
                    nc.scalar.copy(ys[:, n * 512:(n + 1) * 512], p)
                else:
                    nc.vector.tensor_copy(ys[:, n * 512:(n + 1) * 512], p)
                if n == 1:
                    nc.gpsimd.dma_start(y[m * 128:(m + 1) * 128, :], ys)

            # ---- attention ----
            def av(t, ik, nik, pts, o_ps):
                pt, c0 = pts[ik]
                ptg = pt.rearrange("p (h q) -> p h q", q=512)
                for hh in range(2):
                    h = 2 * t + hh
                    nc.tensor.matmul(
                        o_ps[hh][:, c0:512], v_sb[ik][:, h * 65:h * 65 + 65],
                        ptg[:, hh, c0:512],
                        start=(ik == 0), stop=(ik == nik - 1),
                    )

            def attention(t, jq, fills):
                nik = 4 * jq + 4
                o_ps = [
                    ps.tile([65, 512], F32, tag="ot", bufs=2, name=f"ops{t}_{jq}_{_h}")
                    for _h in range(2)
                ]
                pts = {}
                for ik in range(nik):
                    d = ik - 4 * jq
                    c0 = 128 * d if d > 0 else 0   # first potentially-valid column
                    st = ps.tile([128, 1024], F32, tag="st", name=f"st{t}_{jq}_{ik}")
                    stg = st.rearrange("p (h q) -> p h q", q=512)
                    for hh in range(2):
                        r = slice(hh * 64, hh * 64 + 64)
                        nc.tensor.matmul(
                            stg[:, hh, c0:512],
                            kT[t][r, ik * 128:(ik + 1) * 128],
                            qT[t][r, jq * 512 + c0:(jq + 1) * 512],
                            start=True, stop=True,
                        )
                    pt = ptp.tile([128, 1024], BT16, tag="pt", name=f"pt{t}_{jq}_{ik}")
                    ptg = pt.rearrange("p (h q) -> p h q", q=512)
                    if d >= 0:
                        ptm = ptmpp.tile([128, 1024], BT16, tag="ptmp", name=f"ptm{t}_{jq}_{ik}")
                        ptmg = ptm.rearrange("p (h q) -> p h q", q=512)
                        nc.scalar.activation(ptmg[:, :, c0:512], stg[:, :, c0:512], AF.Exp)
                        for hh in range(2):
                            nc.vector.tensor_mul(
                                ptg[:, hh, c0:512],
                                ptmg[:, hh, c0:512],
                                masks_sb[:, d, c0:512],
                            )
                    else:
                        nc.scalar.activation(pt, st, AF.Exp)
                    pts[ik] = (pt, c0)
                    if fills and (ik % 3 == 2 or (jq == 0 and ik >= 1)):
                        fills.popleft()()
                    if ik > 0:
                        av(t, ik - 1, nik, pts, o_ps)
                av(t, nik - 1, nik, pts, o_ps)
                # evict Z row + unnormalized O^T, freeing the PSUM accumulators
                out_h = []
                for hh in range(2):
                    ouz = znp.tile([65, 512], F32, tag="ouz", bufs=6, name=f"oz{t}_{jq}_{hh}")
                    nc.vector.tensor_copy(ouz, o_ps[hh])
                    out_h.append(ouz)
                while fills:
                    fills.popleft()()
                return out_h

            import concourse.bass as bass_mod

            def normalize_a(t, jq, evicted):
                # Stage A: kick off the 1/Z DRAM-broadcast chain. Pack both
                # heads' Z rows [1,512] as [8,64] each -> one [16,64]
                # reciprocal (64 elems/lane), then broadcast 1/Z via a DRAM
                # round-trip (partition-step-0 DMA reads are legal from DRAM).
                zb = znp.tile([16, 64], F32, tag="zb", bufs=2, name=f"zb{t}_{jq}")
                for hh in range(2):
                    ouz = evicted[hh]
                    nc.sync.dma_start(
                        zb[8 * hh:8 * hh + 8, :],
                        ouz[64:65, :].rearrange("o (p q) -> o p q", p=8),
                    )
                rcp = znp.tile([16, 64], F32, tag="rcpb", bufs=2, name=f"rcp{t}_{jq}")
                nc.vector.reciprocal(rcp, zb)
                rcp16 = znp.tile([16, 64], BT16, tag="rcp16b", bufs=2, name=f"rcp16{t}_{jq}")
                nc.vector.tensor_copy(rcp16, rcp)
                nc.sync.dma_start(
                    rcp_dram[jq, 2 * t:2 * t + 2, :].rearrange("h (p q) -> (h p) q", p=8),
                    rcp16,
                )
                bcs = []
                for hh in range(2):
                    bc_sb = znp.tile([64, 512], BT16, tag="bc_sb", bufs=4, name=f"bs{t}_{jq}_{hh}")
                    src = rcp_dram[jq, 2 * t + hh, :]
                    bcast = bass_mod.AP(
                        tensor=src.tensor, offset=src.offset,
                        ap=[[0, 64]] + [list(a) for a in src.ap],
                    )
                    nc.sync.dma_start(bc_sb, bcast)
                    bcs.append(bc_sb)
                return bcs

            def normalize_b(t, jq, evicted, bcs):
                # Stage B (one slot later, after the broadcast landed): scale
                # O^T by 1/Z.
                qs2 = slice(jq * 512, (jq + 1) * 512)
                nc.vector.tensor_mul(oT[t][0:64, qs2], evicted[0][0:64, :], bcs[0])
                tmp = znp.tile([64, 512], BT16, tag="tmp_o", bufs=2, name=f"tm{t}_{jq}")
                nc.vector.tensor_mul(tmp, evicted[1][0:64, :], bcs[1])
                nc.gpsimd.dma_start(oT[t][64:128, qs2], tmp)

            # ---- main schedule ----
            pendA = deque()   # attentions awaiting stage-A normalize
            pendB = deque()   # awaiting stage-B (muls), one slot later
            p3q = deque()
            qk_q(0, 0, xts[0])
            qk_k(0, 0, xts[0])
            xts[1] = dma_xt(1)
            for s in range(3):
                v_unit(0, s, xts[0])
            for jq in range(NJQ):
                for t in range(NP):
                    if t == 1 and jq >= 1 and jq + 1 < NJQ:
                        xts[jq + 1] = dma_xt(jq + 1)
                    fills = deque()
                    if jq == 0 and t == 0:
                        fills.append(lambda: v_unit(0, 3, xts[0]))
                    if t < NP - 1:
                        fills.append(lambda jt=jq, tt=t + 1: qk_q(jt, tt, xts[jt]))
                        fills.append(lambda jt=jq, tt=t + 1: qk_k(jt, tt, xts[jt]))
                    elif jq + 1 < NJQ:
                        fills.append(lambda jt=jq + 1: qk_q(jt, 0, xts[jt]))
                        fills.append(lambda jt=jq + 1: qk_k(jt, 0, xts[jt]))
                    if p3q:
                        m = p3q.popleft()
                        fills.append(lambda mm=m: phase3_n(mm, 0))
                        fills.append(lambda mm=m: phase3_n(mm, 1))
                    if t >= 2 and jq + 1 < NJQ:
                        s0 = 2 * (t - 2)
                        fills.append(lambda jt=jq + 1, s=s0: v_unit(jt, s, xts[jt]))
                        fills.append(lambda jt=jq + 1, s=s0 + 1: v_unit(jt, s, xts[jt]))
                    ev = attention(t, jq, fills)
                    if not (jq == NJQ - 1 and t == NP - 1):
                        # stage A immediately: the broadcast has a full slot
                        # to land before stage B consumes it
                        bcs = normalize_a(t, jq, ev)
                        pendB.append((t, jq, ev, bcs))
                    if len(pendB) >= 2:
                        tb, jb, evb, bcsb = pendB.popleft()
                        normalize_b(tb, jb, evb, bcsb)
                        if tb == NP - 1:
                            p3q.extend(range(4 * jb, 4 * jb + 4))
                    pendA.append((t, jq, ev))

            # ---- tail ----
            # (3,3): 1/Z via SBUF gather + K=1 ones outer-product on the PE —
            # no DRAM round trip. Kick the chain first, then stage-B of (2,3)
            # and the reserved out-proj chunk keep the engines busy under it.
            ta, ja = NP - 1, NJQ - 1
            eva = pendA[-1][2]
            ouz0, ouz1 = eva
            zb = znp.tile([16, 64], F32, tag="zb", bufs=2, name="zb_tail")
            for hh in range(2):
                nc.sync.dma_start(
                    zb[8 * hh:8 * hh + 8, :],
                    eva[hh][64:65, :].rearrange("o (p q) -> o p q", p=8),
                )
            rcp = znp.tile([16, 64], F32, tag="rcpb", bufs=2, name="rcp_tail")
            nc.vector.reciprocal(rcp, zb)
            rcp16 = znp.tile([16, 64], BT16, tag="rcp16b", bufs=2, name="rcp16_tail")
            nc.vector.tensor_copy(rcp16, rcp)
            rcpln = znp.tile([1, 1024], BT16, tag="rcpln", bufs=1, name="rcpln")
            nc.sync.dma_start(
                rcpln.rearrange("o (p q) -> o p q", p=16), rcp16
            )
            tb, jb, evb, bcsb = pendB.popleft()
            normalize_b(tb, jb, evb, bcsb)
            while p3q:     # PE fill while the 1/Z chain completes
                m = p3q.popleft()
                phase3_n(m, 0)
                phase3_n(m, 1)
            bc_pair = ps.tile([128, 1024], F32, tag="st", name="bc_pair")
            nc.tensor.matmul(bc_pair[0:64, 0:512], ones_bf, rcpln[:, 0:512],
                             start=True, stop=True)
            nc.tensor.matmul(bc_pair[0:64, 512:1024], ones_bf, rcpln[:, 512:1024],
                             start=True, stop=True)
            for mi in range(4):
                cs = slice(mi * 128, (mi + 1) * 128)
                gs = slice(ja * 512 + mi * 128, ja * 512 + (mi + 1) * 128)
                nc.vector.tensor_mul(oT[ta][0:64, gs], ouz0[0:64, cs], bc_pair[0:64, cs])
                tmp = znp.tile([64, 128], BT16, tag="tmp_os", bufs=4, name=f"tms{mi}")
                nc.vector.tensor_mul(tmp, ouz1[0:64, cs], bc_pair[0:64, 512 + mi * 128:512 + (mi + 1) * 128])
                nc.sync.dma_start(oT[ta][64:128, gs], tmp)
                phase3_n(4 * ja + mi, 0, alt=True)
                phase3_n(4 * ja + mi, 1, alt=True)

    nc.compile()
    return nc


def _host_prep(x, wq, bq, wk, bk, wv, wo):
    masks_np = np.zeros((128, 4, 512), dtype=BF16)
    qn = np.arange(512)[None, :]
    kn = np.arange(128)[:, None]
    for d in range(4):
        masks_np[:, d, :] = (qn >= kn + 128 * d).astype(BF16)

    per_g = []
    for g in range(G):
        cs = slice(g * HD, (g + 1) * HD)
        per_g.append({
            "wqk": np.ascontiguousarray(
                np.concatenate([wq[:, cs], wk[:, cs]], axis=1)
            ).astype(BF16),
            "wv": np.ascontiguousarray(wv[:, cs]).astype(BF16),
            "wo": np.ascontiguousarray(wo[cs, :]).astype(BF16),
            "bq": np.ascontiguousarray((bq[cs] / 8.0).reshape(NP, 128).T).astype(np.float32),
            "bk": np.ascontiguousarray(bk[cs].reshape(NP, 128).T).astype(np.float32),
            "masks": masks_np,
        })
    in_maps = []
    for c in range(8):
        b, g = divmod(c, G)
        m = dict(per_g[g])
        m["xT"] = np.ascontiguousarray(x[b].T).astype(BF16)
        in_maps.append(m)
    return in_maps


def kernel(x, wq, bq, wk, bk, wv, bv, wo, bo):
    x = np.asarray(x, dtype=np.float32)
    wq = np.asarray(wq, dtype=np.float32)
    bq = np.asarray(bq, dtype=np.float32)
    wk = np.asarray(wk, dtype=np.float32)
    bk = np.asarray(bk, dtype=np.float32)
    wv = np.asarray(wv, dtype=np.float32)
    bv = np.asarray(bv, dtype=np.float32)
    wo = np.asarray(wo, dtype=np.float32)
    bo = np.asarray(bo, dtype=np.float32)

    if "nc" not in _CACHED:
        _CACHED["nc"] = _build()
    nc = _CACHED["nc"]

    in_maps = _host_prep(x, wq, bq, wk, bk, wv, wo)
    res = run_bass_kernel_spmd(nc, in_maps, core_ids=list(range(8)))

    const_row = (bo.astype(np.float64) + bv.astype(np.float64) @ wo.astype(np.float64))
    out = np.empty((B, T, C), dtype=np.float32)
    for b in range(B):
        acc = res.results[2 * b]["y"].astype(np.float64)
        acc += res.results[2 * b + 1]["y"].astype(np.float64)
        acc += const_row[None, :]
        out[b] = acc.astype(np.float32)
    return out


# revision 30
# speedup vs baseline: 1.0458x; 1.0218x over previous
"""Causal self-attention (B=4, T=2048, C=1024, H=16, D=64) on 8 trn2 NeuronCores.

Sharding: core c = (batch b = c//2, head-group g = c%2). Megatron-style within a
batch: each core computes 8 heads' q/k/v (column-parallel) and a row-parallel
partial out-projection. Host sums the two partials per batch and adds the
rank-1 bias term (bo + bv @ wo) -- valid because softmax rows sum to 1, so v's
bias never needs to enter the kernel.

Schedule (v2): fine-grained software pipelining. The q/k projection for pair t
of chunk jq ("qk unit") is issued as PE fill work inside the PREVIOUS pair's
attention (which is ScalarE/exp-bound), v-projection units for chunk jq+1 and
out-projection m-chunks are likewise interleaved as fills at attention tile
milestones. The tail normalize uses a PE K=1 ones-outer-product to broadcast
1/Z (no DRAM round trip), and y is written back in bf16 (summed in f64 host-
side; rel-err impact ~4e-4).
"""
import numpy as np
import ml_dtypes
from collections import deque

import concourse.tile as tile
from concourse import bacc, mybir
from concourse.bass_utils import run_bass_kernel_spmd

BF16 = ml_dtypes.bfloat16
F32 = mybir.dt.float32
BT16 = mybir.dt.bfloat16
AF = mybir.ActivationFunctionType
ALU = mybir.AluOpType

B, T, C, H, D = 4, 2048, 1024, 16, 64
G = 2              # head groups (cores per batch)
HL = H // G        # heads per core = 8
HD = HL * D        # local head dims = 512
NP = 4             # head pairs per core
NJQ = T // 512     # q chunks of 512 = 4
NIK = T // 128     # k tiles of 128 = 16
KC = C // 128      # contraction chunks = 8

_CACHED = {}


def _build():
    nc = bacc.Bacc("TRN2", debug=False)
    # host-prearranged layouts: per-partition-contiguous (8 KB runs, full DMA BW)
    xp = nc.dram_tensor("xp", [NJQ, 128, KC * 512], BT16, kind="ExternalInput").ap()
    wqkp = nc.dram_tensor("wqkp", [128, KC * 1024], BT16, kind="ExternalInput").ap()
    wvp = nc.dram_tensor("wvp", [128, KC * 512], BT16, kind="ExternalInput").ap()
    wo = nc.dram_tensor("wo", [HD, C], BT16, kind="ExternalInput").ap()
    bq = nc.dram_tensor("bq", [128, NP], F32, kind="ExternalInput").ap()
    bk = nc.dram_tensor("bk", [128, NP], F32, kind="ExternalInput").ap()
    masks = nc.dram_tensor("masks", [128, 4, 512], BT16, kind="ExternalInput").ap()
    rcp_dram = nc.dram_tensor("rcp_dram", [NJQ, 8, 512], BT16).ap()
    y = nc.dram_tensor("y", [T, C], BT16, kind="ExternalOutput").ap()

    with tile.TileContext(nc) as tc:
        with (
            tc.tile_pool(name="consts", bufs=1) as consts,
            tc.tile_pool(name="xt", bufs=2) as xtp,
            tc.tile_pool(name="qk", bufs=1) as qkp,
            tc.tile_pool(name="vp", bufs=1) as vp,
            tc.tile_pool(name="otp", bufs=1) as otp,
            tc.tile_pool(name="pt", bufs=6) as ptp,
            tc.tile_pool(name="ptmp", bufs=3) as ptmpp,
            tc.tile_pool(name="zn", bufs=3) as znp,
            tc.tile_pool(name="yst", bufs=4) as ystp,
            tc.tile_pool(name="ps", bufs=2, space="PSUM") as ps,
        ):
            # ---- constants ----
            bq_sb = consts.tile([128, NP], F32, tag="bq")
            nc.sync.dma_start(bq_sb, bq)
            bk_sb = consts.tile([128, NP], F32, tag="bk")
            nc.sync.dma_start(bk_sb, bk)
            masks_sb = consts.tile([128, 4, 512], BT16, tag="masks")
            wqk_sb = consts.tile([128, KC, 2 * HD], BT16, tag="wqk")
            wqk_v = wqkp.rearrange("p (k c) -> p k c", c=1024)

            def dma_xt(jt):
                # x is host-prearranged per-partition-contiguous; mid-kernel
                # chunks stay off the sync queue (reserved for 1/Z chains).
                xt = xtp.tile([128, KC, 512], BT16, tag="xt", bufs=3, name=f"xt{jt}")
                xv = xp[jt].rearrange("p (k c) -> p k c", c=512)
                if jt == 0:
                    for k in range(0, KC, 2):
                        nc.sync.dma_start(xt[:, k:k + 2, :], xv[:, k:k + 2, :])
                else:
                    nc.gpsimd.dma_start(xt[:, 0:4, :], xv[:, 0:4, :])
                    nc.sync.dma_start(xt[:, 4:8, :], xv[:, 4:8, :])
                return xt

            # startup DMA priority across all three DMA queues:
            # sync: xt0 pairs; gpsimd: even wqk pairs; scalar: odd wqk pairs
            dummy_in = consts.tile([1, 16], F32, tag="dummy_in")
            nc.vector.memset(dummy_in, 0.0)
            xts = {0: dma_xt(0)}
            wv_sb = consts.tile([128, KC, HD], BT16, tag="wv")
            wv_v = wvp.rearrange("p (k c) -> p k c", c=512)
            for h in range(0, KC, 4):
                nc.gpsimd.dma_start(wqk_sb[:, h:h + 2, :], wqk_v[:, h:h + 2, :])
                nc.scalar.dma_start(wqk_sb[:, h + 2:h + 4, :], wqk_v[:, h + 2:h + 4, :])
            # preload the Exp activation table while the weight DMAs fly
            dummy_out = consts.tile([1, 16], F32, tag="dummy_out")
            nc.scalar.activation(dummy_out, dummy_in, AF.Exp)
            nc.gpsimd.dma_start(masks_sb, masks)
            nc.gpsimd.dma_start(wv_sb[:, 0:4, :], wv_v[:, 0:4, :])
            nc.scalar.dma_start(wv_sb[:, 4:8, :], wv_v[:, 4:8, :])
            wo_sb = consts.tile([128, NP, C], BT16, tag="wo")
            nc.gpsimd.dma_start(wo_sb, wo.rearrange("(t p) c -> p t c", p=128))
            ones_f = consts.tile([1, 64], F32, tag="ones_f")
            nc.vector.memset(ones_f, 1.0)

            # ---- persistent activations ----
            qT = [qkp.tile([128, T], BT16, tag=f"qT{t}", name=f"qT{t}") for t in range(NP)]
            kT = [qkp.tile([128, T], BT16, tag=f"kT{t}", name=f"kT{t}") for t in range(NP)]
            v_sb = [vp.tile([128, HL * 65], BT16, tag=f"v{i}", name=f"v{i}") for i in range(NIK)]
            oT = [otp.tile([128, T], BT16, tag=f"oT{t}", name=f"oT{t}") for t in range(NP)]

            # ---- phase-1 units ----
            def qk_q(jt, t, xt):
                p = ps.tile([128, 512], F32, tag="fp", bufs=2, name=f"pq{jt}_{t}")
                for k in range(KC):
                    nc.tensor.matmul(
                        p, wqk_sb[:, k, t * 128:(t + 1) * 128], xt[:, k, :],
                        start=(k == 0), stop=(k == KC - 1),
                    )
                if jt == 1 or (jt == 0 and t >= 2):
                    nc.scalar.activation(
                        qT[t][:, jt * 512:(jt + 1) * 512], p,
                        AF.Identity, bias=bq_sb[:, t:t + 1], scale=0.125,
                    )
                else:
                    nc.vector.tensor_scalar(
                        qT[t][:, jt * 512:(jt + 1) * 512], p,
                        0.125, bq_sb[:, t:t + 1], ALU.mult, ALU.add,
                    )

            def qk_k(jt, t, xt):
                p = ps.tile([128, 512], F32, tag="fp", bufs=2, name=f"pk{jt}_{t}")
                for k in range(KC):
                    nc.tensor.matmul(
                        p, wqk_sb[:, k, HD + t * 128:HD + (t + 1) * 128], xt[:, k, :],
                        start=(k == 0), stop=(k == KC - 1),
                    )
                if jt == 1 or (jt == 0 and t >= 2):
                    nc.scalar.activation(
                        kT[t][:, jt * 512:(jt + 1) * 512], p,
                        AF.Identity, bias=bk_sb[:, t:t + 1], scale=1.0,
                    )
                else:
                    nc.vector.tensor_scalar_add(
                        kT[t][:, jt * 512:(jt + 1) * 512], p, bk_sb[:, t:t + 1]
                    )

            def v_unit(jt, s, xt):
                ik = jt * 4 + s
                p = ps.tile([128, 512], F32, tag="fp", bufs=2, name=f"pv{ik}")
                for k in range(KC):
                    nc.tensor.matmul(
                        p, xt[:, k, s * 128:(s + 1) * 128], wv_sb[:, k, :],
                        start=(k == 0), stop=(k == KC - 1),
                    )
                vg = v_sb[ik].rearrange("p (h c) -> p h c", c=65)
                nc.vector.tensor_copy(
                    vg[:, :, 0:64], p.rearrange("p (h c) -> p h c", c=64)
                )
                nc.vector.memset(vg[:, :, 64:65], 1.0)

            # ---- out-projection (m, n) sub-chunk; one merged y DMA per m ----
            ys_tiles = {}

            def phase3_n(m, n, alt=False):
                p = ps.tile([128, 512], F32, tag="fp", bufs=2, name=f"py{m}_{n}")
                for t in range(NP):
                    nc.tensor.matmul(
                        p, oT[t][:, m * 128:(m + 1) * 128],
                        wo_sb[:, t, n * 512:(n + 1) * 512],
                        start=(t == 0), stop=(t == NP - 1),
                    )
                if n == 0:
                    ys_tiles[m] = ystp.tile([128, 1024], BT16, tag="y", name=f"ys{m}")
                ys = ys_tiles[m]
                if alt:
                    nc.scalar.copy(ys[:, n * 512:(n + 1) * 512], p)
                else:
                    nc.vector.tensor_copy(ys[:, n * 512:(n + 1) * 512], p)
                if n == 1:
                    eng = nc.sync if (alt and m % 2 == 1) else nc.gpsimd
                    eng.dma_start(y[m * 128:(m + 1) * 128, :], ys)

            # ---- attention ----
            def av(t, ik, nik, pts, o_ps):
                pt, c0 = pts[ik]
                ptg = pt.rearrange("p (h q) -> p h q", q=512)
                for hh in range(2):
                    h = 2 * t + hh
                    nc.tensor.matmul(
                        o_ps[hh][:, c0:512], v_sb[ik][:, h * 65:h * 65 + 65],
                        ptg[:, hh, c0:512],
                        start=(ik == 0), stop=(ik == nik - 1),
                    )

            def attention(t, jq, fills, evict_split=False):
                nik = 4 * jq + 4
                o_ps = [
                    ps.tile([65, 512], F32, tag="ot", bufs=2, name=f"ops{t}_{jq}_{_h}")
                    for _h in range(2)
                ]
                pts = {}
                for ik in range(nik):
                    d = ik - 4 * jq
                    c0 = 128 * d if d > 0 else 0   # first potentially-valid column
                    st = ps.tile([128, 1024], F32, tag="st", name=f"st{t}_{jq}_{ik}")
                    stg = st.rearrange("p (h q) -> p h q", q=512)
                    for hh in range(2):
                        r = slice(hh * 64, hh * 64 + 64)
                        nc.tensor.matmul(
                            stg[:, hh, c0:512],
                            kT[t][r, ik * 128:(ik + 1) * 128],
                            qT[t][r, jq * 512 + c0:(jq + 1) * 512],
                            start=True, stop=True,
                        )
                    pt = ptp.tile([128, 1024], BT16, tag="pt", name=f"pt{t}_{jq}_{ik}")
                    ptg = pt.rearrange("p (h q) -> p h q", q=512)
                    if d >= 0:
                        ptm = ptmpp.tile([128, 1024], BT16, tag="ptmp", name=f"ptm{t}_{jq}_{ik}")
                        ptmg = ptm.rearrange("p (h q) -> p h q", q=512)
                        nc.scalar.activation(ptmg[:, :, c0:512], stg[:, :, c0:512], AF.Exp)
                        for hh in range(2):
                            nc.vector.tensor_mul(
                                ptg[:, hh, c0:512],
                                ptmg[:, hh, c0:512],
                                masks_sb[:, d, c0:512],
                            )
                    else:
                        nc.scalar.activation(pt, st, AF.Exp)
                    pts[ik] = (pt, c0)
                    if fills and (ik % 3 == 2 or (jq == 0 and ik >= 1)):
                        fills.popleft()()
                    if ik > 0:
                        av(t, ik - 1, nik, pts, o_ps)
                av(t, nik - 1, nik, pts, o_ps)
                # evict Z row + unnormalized O^T, freeing the PSUM accumulators
                out_h = []
                for hh in range(2):
                    ouz = znp.tile([65, 512], F32, tag="ouz", bufs=6, name=f"oz{t}_{jq}_{hh}")
                    if evict_split and hh == 0:
                        nc.scalar.copy(ouz, o_ps[hh])
                    else:
                        nc.vector.tensor_copy(ouz, o_ps[hh])
                    out_h.append(ouz)
                while fills:
                    fills.popleft()()
                return out_h

            import concourse.bass as bass_mod

            def normalize_a(t, jq, evicted):
                # Stage A: kick off the 1/Z DRAM-broadcast chain. Pack both
                # heads' Z rows [1,512] as [8,64] each -> one [16,64]
                # reciprocal (64 elems/lane), then broadcast 1/Z via a DRAM
                # round-trip (partition-step-0 DMA reads are legal from DRAM).
                zb = znp.tile([16, 64], F32, tag="zb", bufs=2, name=f"zb{t}_{jq}")
                for hh in range(2):
                    ouz = evicted[hh]
                    nc.sync.dma_start(
                        zb[8 * hh:8 * hh + 8, :],
                        ouz[64:65, :].rearrange("o (p q) -> o p q", p=8),
                    )
                rcp = znp.tile([16, 64], F32, tag="rcpb", bufs=2, name=f"rcp{t}_{jq}")
                nc.vector.reciprocal(rcp, zb)
                rcp16 = znp.tile([16, 64], BT16, tag="rcp16b", bufs=2, name=f"rcp16{t}_{jq}")
                nc.vector.tensor_copy(rcp16, rcp)
                nc.sync.dma_start(
                    rcp_dram[jq, 2 * t:2 * t + 2, :].rearrange("h (p q) -> (h p) q", p=8),
                    rcp16,
                )
                bcs = []
                for hh in range(2):
                    bc_sb = znp.tile([64, 512], BT16, tag="bc_sb", bufs=6, name=f"bs{t}_{jq}_{hh}")
                    src = rcp_dram[jq, 2 * t + hh, :]
                    bcast = bass_mod.AP(
                        tensor=src.tensor, offset=src.offset,
                        ap=[[0, 64]] + [list(a) for a in src.ap],
                    )
                    nc.sync.dma_start(bc_sb, bcast)
                    bcs.append(bc_sb)
                return bcs

            def normalize_b(t, jq, evicted, bcs):
                # Stage B (one slot later, after the broadcast landed): scale
                # O^T by 1/Z.
                qs2 = slice(jq * 512, (jq + 1) * 512)
                nc.vector.tensor_mul(oT[t][0:64, qs2], evicted[0][0:64, :], bcs[0])
                tmp = znp.tile([64, 512], BT16, tag="tmp_o", bufs=2, name=f"tm{t}_{jq}")
                nc.vector.tensor_mul(tmp, evicted[1][0:64, :], bcs[1])
                nc.gpsimd.dma_start(oT[t][64:128, qs2], tmp)

            # ---- main schedule ----
            pendA = deque()   # attentions awaiting stage-A normalize
            pendB = deque()   # awaiting stage-B (muls), one slot later
            p3q = deque()
            qk_q(0, 0, xts[0])
            qk_k(0, 0, xts[0])
            xts[1] = dma_xt(1)
            for s in range(3):
                v_unit(0, s, xts[0])
            for jq in range(NJQ):
                for t in range(NP):
                    if t == 1 and jq >= 1 and jq + 1 < NJQ:
                        xts[jq + 1] = dma_xt(jq + 1)
                    fills = deque()
                    if jq == 0 and t == 0:
                        fills.append(lambda: v_unit(0, 3, xts[0]))
                    if t < NP - 1:
                        fills.append(lambda jt=jq, tt=t + 1: qk_q(jt, tt, xts[jt]))
                        fills.append(lambda jt=jq, tt=t + 1: qk_k(jt, tt, xts[jt]))
                    elif jq + 1 < NJQ:
                        fills.append(lambda jt=jq + 1: qk_q(jt, 0, xts[jt]))
                        fills.append(lambda jt=jq + 1: qk_k(jt, 0, xts[jt]))
                    if p3q:
                        m = p3q.popleft()
                        fills.append(lambda mm=m: phase3_n(mm, 0))
                        fills.append(lambda mm=m: phase3_n(mm, 1))
                    if t >= 2 and jq + 1 < NJQ:
                        s0 = 2 * (t - 2)
                        fills.append(lambda jt=jq + 1, s=s0: v_unit(jt, s, xts[jt]))
                        fills.append(lambda jt=jq + 1, s=s0 + 1: v_unit(jt, s, xts[jt]))
                    ev = attention(t, jq, fills,
                                   evict_split=(jq == NJQ - 1 and t == NP - 1))
                    if not (jq == NJQ - 1 and t == NP - 1):
                        # stage A immediately: the broadcast has two slots
                        # to land before stage B consumes it
                        bcs = normalize_a(t, jq, ev)
                        pendB.append((t, jq, ev, bcs))
                    if len(pendB) >= 3:
                        tb, jb, evb, bcsb = pendB.popleft()
                        normalize_b(tb, jb, evb, bcsb)
                        if tb == NP - 1:
                            p3q.extend(range(4 * jb, 4 * jb + 4))
                    pendA.append((t, jq, ev))

            # ---- tail ----
            # (3,3): 1/Z via SBUF gather + K=1 ones outer-product on the PE —
            # no DRAM round trip. Kick the chain first; the two remaining
            # stage-Bs and reserved out-proj chunks run under it.
            ta, ja = NP - 1, NJQ - 1
            eva = pendA[-1][2]
            ouz0, ouz1 = eva
            zb = znp.tile([16, 64], F32, tag="zb", bufs=2, name="zb_tail")
            for hh in range(2):
                nc.sync.dma_start(
                    zb[8 * hh:8 * hh + 8, :],
                    eva[hh][64:65, :].rearrange("o (p q) -> o p q", p=8),
                )
            rcp = znp.tile([16, 64], F32, tag="rcpb", bufs=2, name="rcp_tail")
            nc.vector.reciprocal(rcp, zb)
            rcpln = znp.tile([1, 1024], F32, tag="rcpln", bufs=1, name="rcpln")
            nc.sync.dma_start(
                rcpln.rearrange("o (p q) -> o p q", p=16), rcp
            )
            while pendB:
                tb, jb, evb, bcsb = pendB.popleft()
                normalize_b(tb, jb, evb, bcsb)
            while p3q:     # PE fill while the 1/Z chain completes
                m = p3q.popleft()
                phase3_n(m, 0)
                phase3_n(m, 1)
            bc_pair = ps.tile([128, 1024], F32, tag="st", name="bc_pair")
            nc.tensor.matmul(bc_pair[0:64, 0:512], ones_f, rcpln[:, 0:512],
                             start=True, stop=True)
            nc.tensor.matmul(bc_pair[0:64, 512:1024], ones_f, rcpln[:, 512:1024],
                             start=True, stop=True)
            for mi in range(4):
                cs = slice(mi * 128, (mi + 1) * 128)
                gs = slice(ja * 512 + mi * 128, ja * 512 + (mi + 1) * 128)
                nc.vector.tensor_mul(oT[ta][0:64, gs], ouz0[0:64, cs], bc_pair[0:64, cs])
                tmp = znp.tile([64, 128], BT16, tag="tmp_os", bufs=4, name=f"tms{mi}")
                nc.vector.tensor_mul(tmp, ouz1[0:64, cs], bc_pair[0:64, 512 + mi * 128:512 + (mi + 1) * 128])
                nc.sync.dma_start(oT[ta][64:128, gs], tmp)
                phase3_n(4 * ja + mi, 0, alt=True)
                phase3_n(4 * ja + mi, 1, alt=True)

    nc.compile()
    return nc


def _host_prep(x, wq, bq, wk, bk, wv, wo):
    masks_np = np.zeros((128, 4, 512), dtype=BF16)
    qn = np.arange(512)[None, :]
    kn = np.arange(128)[:, None]
    for d in range(4):
        masks_np[:, d, :] = (qn >= kn + 128 * d).astype(BF16)

    per_g = []
    for g in range(G):
        cs = slice(g * HD, (g + 1) * HD)
        wqk_g = np.concatenate([wq[:, cs], wk[:, cs]], axis=1)      # [C, 1024]
        wqkp = wqk_g.reshape(KC, 128, 1024).transpose(1, 0, 2).reshape(128, KC * 1024)
        wv_g = wv[:, cs]                                            # [C, 512]
        wvp = wv_g.reshape(KC, 128, 512).transpose(1, 0, 2).reshape(128, KC * 512)
        per_g.append({
            "wqkp": np.ascontiguousarray(wqkp).astype(BF16),
            "wvp": np.ascontiguousarray(wvp).astype(BF16),
            "wo": np.ascontiguousarray(wo[cs, :]).astype(BF16),
            "bq": np.ascontiguousarray((bq[cs] / 8.0).reshape(NP, 128).T).astype(np.float32),
            "bk": np.ascontiguousarray(bk[cs].reshape(NP, 128).T).astype(np.float32),
            "masks": masks_np,
        })
    in_maps = []
    xps = []
    for b in range(B):
        xT = x[b].T                                                 # [C, T]
        xpb = (xT.reshape(KC, 128, NJQ, 512).transpose(2, 1, 0, 3)
               .reshape(NJQ, 128, KC * 512))
        xps.append(np.ascontiguousarray(xpb).astype(BF16))
    for c in range(8):
        b, g = divmod(c, G)
        m = dict(per_g[g])
        m["xp"] = xps[b]
        in_maps.append(m)
    return in_maps


def kernel(x, wq, bq, wk, bk, wv, bv, wo, bo):
    x = np.asarray(x, dtype=np.float32)
    wq = np.asarray(wq, dtype=np.float32)
    bq = np.asarray(bq, dtype=np.float32)
    wk = np.asarray(wk, dtype=np.float32)
    bk = np.asarray(bk, dtype=np.float32)
    wv = np.asarray(wv, dtype=np.float32)
    bv = np.asarray(bv, dtype=np.float32)
    wo = np.asarray(wo, dtype=np.float32)
    bo = np.asarray(bo, dtype=np.float32)

    if "nc" not in _CACHED:
        _CACHED["nc"] = _build()
    nc = _CACHED["nc"]

    in_maps = _host_prep(x, wq, bq, wk, bk, wv, wo)
    res = run_bass_kernel_spmd(nc, in_maps, core_ids=list(range(8)))

    const_row = (bo.astype(np.float64) + bv.astype(np.float64) @ wo.astype(np.float64))
    out = np.empty((B, T, C), dtype=np.float32)
    for b in range(B):
        acc = res.results[2 * b]["y"].astype(np.float64)
        acc += res.results[2 * b + 1]["y"].astype(np.float64)
        acc += const_row[None, :]
        out[b] = acc.astype(np.float32)
    return out
